# revision 16
# baseline (speedup 1.0000x reference)
"""Trainium2 Bass kernel for nn_DecoderLayer (dense transformer decoder layer).

Strategy (8 NeuronCores, full inputs in / full output out):
  - core c handles batch b = c//4 and query-quarter r = c%4 (rows [r*S/4, (r+1)*S/4)).
  - Activations are kept TRANSPOSED on-chip (x^T [D, n]) so every projection
    matmul runs with the contraction dim on partitions and fp32r (fast fp32)
    streaming at full rate with N=512 moving columns.
  - Attention per head: S^T[k, q] = K_h^T-slices.T @ Q_h^T (scores transposed),
    exp on the ACT engine (1/8 scale fused), causal/arbitrary q-k mask applied
    as data-driven multiplicative [128, W] tiles (uniform SPMD program, per-core
    mask DATA), softmax denominators obtained free by augmenting V with a ones
    column, normalization deferred to a per-head [64, W] multiply.
  - K/V are computed for the full batch (cheap, redundant across the 4 cores of
    a batch group); Q/out-proj/LayerNorm/FFN are sharded by query rows.
  - The single collective: AllGather of x1 (post-LN1) within each 4-core batch
    group, needed because cross-attention K2/V2 are projections of full x1.
  - LayerNorm runs in transposed layout: cross-partition sums via ones-matmul
    on the PE, stats broadcast back to [128, W] via ones-matmul.
"""

import sys

if "/opt/trn_rl_repo" not in sys.path:
    sys.path.insert(0, "/opt/trn_rl_repo")

import numpy as np

P = 128
HD = 64
HD1 = HD + 1
EPS = 1e-5


class Cfg:
    def __init__(self, B=2, S=2048, D=1024, H=16, DFF=4096, use_collective=True):
        self.B, self.S, self.D, self.H, self.DFF = B, S, D, H, DFF
        self.W = S // 4            # local query rows per core
        self.DT = D // P           # feature-dim tiles
        self.NT = S // P           # sequence tiles (keys)
        self.FT = DFF // P         # ffn hidden tiles
        self.HP = P // HD          # heads per partition-tile (2)
        self.NCH = max(1, S // 512)   # n-chunks for K-orientation matmuls
        self.NCW = S // self.NCH      # n-chunk width (<=512)
        self.VCW = min(512, D)        # v-dout chunk width
        self.VCN = D // self.VCW
        self.KTG = min(4, self.NT)    # k-tiles per exp group
        self.NG = self.NT // self.KTG
        self.use_collective = use_collective
        assert D == H * HD
        assert self.W % P == 0 and D % P == 0 and DFF % P == 0 and S % P == 0


class Flags:
    def __init__(self):
        self.qkb1 = self.vb1 = self.ob1 = False
        self.qkb2 = self.vb2 = self.ob2 = False
        self.fb1 = self.fb2 = False
        self.g1 = self.b1 = self.g2 = self.b2 = self.g3 = self.b3 = False
        self.m1 = True      # trg mask multiplicative tiles
        self.kb2 = False    # enc mask additive per-k bias


def _build(nc, tc, cfg, fl):
    import concourse.bass as bass
    import concourse.mybir as mybir
    import concourse.tile as tile  # noqa: F401
    from contextlib import ExitStack

    AF = mybir.ActivationFunctionType
    f32 = mybir.dt.float32
    f32r = mybir.dt.float32r

    def r32(ap):
        return ap.bitcast(f32r)

    B, S, D, H, DFF = cfg.B, cfg.S, cfg.D, cfg.H, cfg.DFF
    W, DT, NT, FT, HP = cfg.W, cfg.DT, cfg.NT, cfg.FT, cfg.HP
    NCH, NCW, VCW, VCN = cfg.NCH, cfg.NCW, cfg.VCW, cfg.VCN
    KTG, NG = cfg.KTG, cfg.NG
    HPC = VCW // HD  # heads per v-chunk

    # ---------------- DRAM parameters ----------------
    def din(name, shape):
        return nc.dram_tensor(name, shape, f32, kind="ExternalInput").ap()

    xT = din("xT", [D, S])
    xTl = din("xTl", [D, W])
    qkvwT1 = din("qkvwT1", [D, 3 * D])
    qkvwT2 = din("qkvwT2", [D, 3 * D])
    owT1 = din("owT1", [D, D])
    owT2 = din("owT2", [D, D])
    w1T = din("w1T", [D, DFF])
    w2T = din("w2T", [DFF, D])
    m1 = din("m1", [NT, P, W]) if fl.m1 else None
    kb2 = din("kb2", [NT, P, 1]) if fl.kb2 else None
    qkvb1 = din("qkvb1", [3 * D]) if fl.qkb1 else None
    qkvb2 = din("qkvb2", [3 * D]) if fl.qkb2 else None
    vb1 = din("vb1", [P, D]) if fl.vb1 else None
    vb2 = din("vb2", [P, D]) if fl.vb2 else None
    ob1 = din("ob1", [D]) if fl.ob1 else None
    ob2 = din("ob2", [D]) if fl.ob2 else None
    fb1d = din("fb1", [DFF]) if fl.fb1 else None
    fb2d = din("fb2", [D]) if fl.fb2 else None
    lnp = {}
    for nm, use in [("g1", fl.g1), ("b1", fl.b1), ("g2", fl.g2),
                    ("b2", fl.b2), ("g3", fl.g3), ("b3", fl.b3)]:
        lnp[nm] = din(nm, [D]) if use else None
    out = nc.dram_tensor("out", [D, W], f32, kind="ExternalOutput").ap()

    es = ExitStack()
    with es:
        dramp = es.enter_context(tc.tile_pool(name="dram", bufs=1, space="DRAM"))
        kT1s = dramp.tile([D, S], f32)
        v1s = dramp.tile([NT, P, H, HD1], f32)
        kT2s = dramp.tile([D, S], f32)
        v2s = dramp.tile([NT, P, H, HD1], f32)
        if cfg.use_collective:
            xb = dramp.tile([D, W], f32)
            agx = dramp.tile([4 * D, W], f32)

        const = es.enter_context(tc.tile_pool(name="const", bufs=1))
        ones_p1 = const.tile([P, 1], f32)
        nc.vector.memset(ones_p1[:, :], 1.0)
        ones_1p = const.tile([1, P], f32)
        nc.vector.memset(ones_1p[0:1, :], 1.0)
        eps_t = const.tile([1, 1], f32)
        nc.vector.memset(eps_t[0:1, :], EPS)

        def ldvec(dram_vec, n_tiles, name):
            """[D]-style vector -> [P, n_tiles] sbuf tile (per-partition slices)."""
            t = const.tile([P, n_tiles], f32, tag=name)
            nc.sync.dma_start(
                out=t[:, :],
                in_=dram_vec.rearrange("(t p) -> p t", p=P),
            )
            return t

        qkb1sb = ldvec(qkvb1[0 : 2 * D], 2 * DT, "qkb1") if fl.qkb1 else None
        qkb2sb = ldvec(qkvb2[0 : 2 * D], 2 * DT, "qkb2") if fl.qkb2 else None
        ob1sb = ldvec(ob1, DT, "ob1") if fl.ob1 else None
        ob2sb = ldvec(ob2, DT, "ob2") if fl.ob2 else None
        fb1sb = ldvec(fb1d, FT, "fb1") if fl.fb1 else None
        fb2sb = ldvec(fb2d, DT, "fb2") if fl.fb2 else None
        lns = {k: (ldvec(v, DT, "ln" + k) if v is not None else None)
               for k, v in lnp.items()}
        vb1sb = None
        if fl.vb1:
            vb1sb = const.tile([P, D], f32, tag="vb1")
            nc.sync.dma_start(out=vb1sb[:, :], in_=vb1[:, :])
        vb2sb = None
        if fl.vb2:
            vb2sb = const.tile([P, D], f32, tag="vb2")
            nc.sync.dma_start(out=vb2sb[:, :], in_=vb2[:, :])
        kb2sb = None
        if fl.kb2:
            kb2sb = const.tile([P, NT], f32, tag="kb2")
            nc.sync.dma_start(out=kb2sb[:, :], in_=kb2.rearrange("n p o -> p (n o)"))

        pers = es.enter_context(tc.tile_pool(name="pers", bufs=1))
        qT = pers.tile([P, DT, W], f32)      # Q^T local (reused block2)
        aoT = pers.tile([P, DT, W], f32)     # attention out^T (reused)
        x1T = pers.tile([P, DT, W], f32)     # x1 local

        # =========== QKV projection phase ===========
        def qkv_phase(xTfull, wT, kTs, vs, qkb, vbsb, x_is_sbuf):
            """xTfull: SBUF tile [P, DT, S] (or accessor fn) holding x^T full batch.
            Writes K^T -> kTs dram, V(+ones) -> vs dram, Q^T local -> qT sbuf."""
            with tc.tile_pool(name="qkv_w", bufs=3) as wp, \
                 tc.tile_pool(name="qkv_wv", bufs=1) as wvp, \
                 tc.tile_pool(name="qkv_st", bufs=2) as stp, \
                 tc.tile_pool(name="qkv_ps", bufs=1, space="PSUM") as psp, \
                 tc.tile_pool(name="qkv_psq", bufs=2, space="PSUM") as psq:
                # ---- K^T [D, S] ----
                for dk in range(DT):
                    ps = psp.tile([P, NCH, NCW], f32, tag="kps")
                    for dt in range(DT):
                        wsb = wp.tile([P, P], f32, tag="wk")
                        nc.sync.dma_start(
                            out=r32(wsb[:, :]),
                            in_=r32(wT[dt * P : (dt + 1) * P, D + dk * P : D + (dk + 1) * P]),
                        )
                        for nch in range(NCH):
                            nc.tensor.matmul(
                                ps[:, nch, :],
                                lhsT=r32(wsb[:, :]),
                                rhs=r32(xTfull[:, dt, nch * NCW : (nch + 1) * NCW]),
                                start=(dt == 0),
                                stop=(dt == DT - 1),
                            )
                    st = stp.tile([P, NCH, NCW], f32, tag="kst")
                    if qkb is not None:
                        nc.scalar.activation(
                            out=st[:, :, :], in_=ps[:, :, :], func=AF.Identity,
                            bias=qkb[:, DT + dk : DT + dk + 1], scale=1.0,
                        )
                    else:
                        nc.scalar.activation(
                            out=st[:, :, :], in_=ps[:, :, :], func=AF.Copy,
                        )
                    nc.sync.dma_start(
                        out=kTs[dk * P : (dk + 1) * P, :],
                        in_=st[:, :, :].rearrange("p a b -> p (a b)"),
                    )
                # ---- V natural [n, dout] + ones column ----
                for vc in range(VCN):
                    wv = wvp.tile([P, DT, VCW], f32, tag="wv")
                    nc.sync.dma_start(
                        out=r32(wv[:, :, :]),
                        in_=r32(wT[:, 2 * D + vc * VCW : 2 * D + (vc + 1) * VCW]
                                .rearrange("(t p) v -> p t v", p=P)),
                    )
                    for nt in range(NT):
                        ps = psq.tile([P, VCW], f32, tag="vps")
                        for dt in range(DT):
                            nc.tensor.matmul(
                                ps[:, :],
                                lhsT=r32(xTfull[:, dt, nt * P : (nt + 1) * P]),
                                rhs=r32(wv[:, dt, :]),
                                start=(dt == 0),
                                stop=(dt == DT - 1),
                            )
                        st = stp.tile([P, HPC, HD1], f32, tag="vst")
                        nc.scalar.activation(
                            out=st[:, :, 0:HD],
                            in_=ps.rearrange("p (h d) -> p h d", d=HD),
                            func=AF.Copy,
                        )
                        if vbsb is not None:
                            nc.vector.tensor_add(
                                st[:, :, 0:HD],
                                st[:, :, 0:HD],
                                vbsb[:, vc * VCW : (vc + 1) * VCW].rearrange(
                                    "p (h d) -> p h d", d=HD),
                            )
                        nc.vector.memset(st[:, :, HD:HD1], 1.0)
                        nc.sync.dma_start(
                            out=vs[nt, :, vc * HPC : (vc + 1) * HPC, :],
                            in_=st[:, :, :],
                        )
                # ---- Q^T local [D, W] ----
                for dq in range(DT):
                    ps = psq.tile([P, W], f32, tag="qps")
                    for dt in range(DT):
                        wsb = wp.tile([P, P], f32, tag="wq")
                        nc.sync.dma_start(
                            out=r32(wsb[:, :]),
                            in_=r32(wT[dt * P : (dt + 1) * P, dq * P : (dq + 1) * P]),
                        )
                        nc.tensor.matmul(
                            ps[:, :],
                            lhsT=r32(wsb[:, :]),
                            rhs=r32(x_is_sbuf[:, dt, :]),
                            start=(dt == 0),
                            stop=(dt == DT - 1),
                        )
                    if qkb is not None:
                        nc.scalar.activation(
                            out=r32(qT[:, dq, :]), in_=ps[:, :], func=AF.Identity,
                            bias=qkb[:, dq : dq + 1], scale=1.0,
                        )
                    else:
                        nc.scalar.activation(
                            out=r32(qT[:, dq, :]), in_=ps[:, :], func=AF.Copy,
                        )

        # =========== attention phase ===========
        def attn_phase(kTs, vs, m1sb, use_kb2):
            with tc.tile_pool(name="at_kv", bufs=2) as kvp, \
                 tc.tile_pool(name="at_ex", bufs=2) as exp_, \
                 tc.tile_pool(name="at_dn", bufs=2) as dnp, \
                 tc.tile_pool(name="at_ps", bufs=1, space="PSUM") as psp, \
                 tc.tile_pool(name="at_po", bufs=2, space="PSUM") as pop:
                for h in range(H):
                    hh = (h % HP) * HD  # partition base shared with q_h
                    k_h = kvp.tile([P, S], f32, tag="kh")
                    nc.sync.dma_start(
                        out=r32(k_h[hh : hh + HD, :]),
                        in_=r32(kTs[h * HD : (h + 1) * HD, :]),
                    )
                    v_h = kvp.tile([P, NT, HD1], f32, tag="vh")
                    nc.sync.dma_start(
                        out=r32(v_h[:, :, :]),
                        in_=r32(vs[:, :, h, :].rearrange("n p d -> p n d")),
                    )
                    q_h = qT[hh : hh + HD, h // HP, :]
                    po = pop.tile([P, W], f32, tag="po")
                    for g in range(NG):
                        ps = psp.tile([P, KTG, W], f32, tag="sc")
                        for o in range(KTG):
                            kt = g * KTG + o
                            nc.tensor.matmul(
                                ps[:, o, :],
                                lhsT=r32(k_h[hh : hh + HD, kt * P : (kt + 1) * P]),
                                rhs=r32(q_h),
                                start=True,
                                stop=True,
                            )
                        ex = exp_.tile([P, KTG, W], f32, tag="ex")
                        if use_kb2:
                            for o in range(KTG):
                                kt = g * KTG + o
                                nc.scalar.activation(
                                    out=r32(ex[:, o, :]), in_=ps[:, o, :], func=AF.Exp,
                                    bias=kb2sb[:, kt : kt + 1], scale=1.0 / np.sqrt(HD),
                                )
                        else:
                            nc.scalar.activation(
                                out=r32(ex[:, :, :]), in_=ps[:, :, :], func=AF.Exp,
                                scale=1.0 / np.sqrt(HD),
                            )
                        if m1sb is not None:
                            for o in range(KTG):
                                kt = g * KTG + o
                                nc.vector.tensor_mul(
                                    r32(ex[:, o, :]), ex[:, o, :], m1sb[:, kt, :]
                                )
                        for o in range(KTG):
                            kt = g * KTG + o
                            nc.tensor.matmul(
                                po[0:HD1, :],
                                lhsT=r32(v_h[:, kt, :]),
                                rhs=r32(ex[:, o, :]),
                                start=(g == 0 and o == 0),
                                stop=(g == NG - 1 and o == KTG - 1),
                            )
                    dinv = dnp.tile([1, W], f32, tag="dinv")
                    nc.vector.reciprocal(dinv[0:1, :], po[HD:HD1, :])
                    dinvb = dnp.tile([HD, W], f32, tag="dinvb")
                    nc.gpsimd.partition_broadcast(
                        dinvb[0:HD, :], dinv[0:1, :], channels=HD
                    )
                    nc.vector.tensor_mul(
                        r32(aoT[hh : hh + HD, h // HP, :]),
                        po[0:HD, :],
                        dinvb[0:HD, :],
                    )

        # =========== layernorm (transposed layout) ===========
        def ln_t(pre, out_t, g_sb, b_sb, lpp, lp, round_out=True):
            ro = r32 if round_out else (lambda ap: ap)
            acc = lp.tile([P, W], f32, tag="lnacc")
            nc.vector.tensor_add(acc[:, :], pre[:, 0, :], pre[:, 1, :])
            for d in range(2, DT):
                nc.vector.tensor_add(acc[:, :], acc[:, :], pre[:, d, :])
            sqa = lp.tile([P, W], f32, tag="lnsqa")
            nc.scalar.square(sqa[:, :], pre[:, 0, :])
            for d in range(1, DT):
                sqt = lp.tile([P, W], f32, tag="lnsqt")
                nc.scalar.square(sqt[:, :], pre[:, d, :])
                nc.vector.tensor_add(sqa[:, :], sqa[:, :], sqt[:, :])
            sums = lpp.tile([1, W], f32, tag="lnsums")
            nc.tensor.matmul(sums[0:1, :], lhsT=ones_p1[:, :],
                             rhs=acc[:, :], start=True, stop=True)
            sqs = lpp.tile([1, W], f32, tag="lnsqs")
            nc.tensor.matmul(sqs[0:1, :], lhsT=ones_p1[:, :],
                             rhs=sqa[:, :], start=True, stop=True)
            mu = lp.tile([1, W], f32, tag="lnmu")
            nc.vector.tensor_scalar_mul(mu[0:1, :], sums[0:1, :], 1.0 / D)
            ex2 = lp.tile([1, W], f32, tag="lnex2")
            nc.vector.tensor_scalar_mul(ex2[0:1, :], sqs[0:1, :], 1.0 / D)
            mu2 = lp.tile([1, W], f32, tag="lnmu2")
            nc.scalar.square(mu2[0:1, :], mu[0:1, :])
            var = lp.tile([1, W], f32, tag="lnvar")
            nc.vector.tensor_sub(var[0:1, :], ex2[0:1, :], mu2[0:1, :])
            sd = lp.tile([1, W], f32, tag="lnsd")
            nc.scalar.activation(out=sd[0:1, :], in_=var[0:1, :], func=AF.Sqrt,
                                 bias=eps_t[0:1, :], scale=1.0)
            rstd = lp.tile([1, W], f32, tag="lnrstd")
            nc.vector.reciprocal(rstd[0:1, :], sd[0:1, :])
            mub = lpp.tile([P, W], f32, tag="lnmub")
            nc.tensor.matmul(mub[:, :], lhsT=ones_1p[0:1, :],
                             rhs=mu[0:1, :], start=True, stop=True)
            rstdb = lpp.tile([P, W], f32, tag="lnrstdb")
            nc.tensor.matmul(rstdb[:, :], lhsT=ones_1p[0:1, :],
                             rhs=rstd[0:1, :], start=True, stop=True)
            for d in range(DT):
                t1 = lp.tile([P, W], f32, tag="lnt1")
                nc.vector.tensor_sub(t1[:, :], pre[:, d, :], mub[:, :])
                nc.vector.tensor_mul(ro(out_t[:, d, :]), t1[:, :], rstdb[:, :])
                if g_sb is not None:
                    nc.vector.tensor_scalar_mul(
                        ro(out_t[:, d, :]), out_t[:, d, :], g_sb[:, d : d + 1])
                if b_sb is not None:
                    nc.vector.tensor_scalar_add(
                        ro(out_t[:, d, :]), out_t[:, d, :], b_sb[:, d : d + 1])

        # =========== out-projection + residual + LN ===========
        def proj_resid_ln(owT, obsb, residT, g_sb, b_sb, out_t):
            with tc.tile_pool(name="pr_w", bufs=3) as wp, \
                 tc.tile_pool(name="pr_t", bufs=2) as lp, \
                 tc.tile_pool(name="pr_pre", bufs=1) as prep, \
                 tc.tile_pool(name="pr_ps", bufs=2, space="PSUM") as psp, \
                 tc.tile_pool(name="pr_lnps", bufs=1, space="PSUM") as lpp:
                pre = prep.tile([P, DT, W], f32, tag="pre")
                for d in range(DT):
                    ps = psp.tile([P, W], f32, tag="prps")
                    for dt in range(DT):
                        wsb = wp.tile([P, P], f32, tag="prw")
                        nc.sync.dma_start(
                            out=r32(wsb[:, :]),
                            in_=r32(owT[dt * P : (dt + 1) * P, d * P : (d + 1) * P]),
                        )
                        nc.tensor.matmul(
                            ps[:, :], lhsT=r32(wsb[:, :]), rhs=r32(aoT[:, dt, :]),
                            start=(dt == 0), stop=(dt == DT - 1),
                        )
                    if obsb is not None:
                        tmp = lp.tile([P, W], f32, tag="prtmp")
                        nc.scalar.activation(out=tmp[:, :], in_=ps[:, :],
                                             func=AF.Identity,
                                             bias=obsb[:, d : d + 1], scale=1.0)
                        nc.vector.tensor_add(pre[:, d, :], tmp[:, :],
                                             residT[:, d, :])
                    else:
                        nc.vector.tensor_add(pre[:, d, :], ps[:, :],
                                             residT[:, d, :])
                ln_t(pre, out_t, g_sb, b_sb, lpp, lp)

        # ================= pipeline =================
        with tc.tile_pool(name="xtl", bufs=1) as xtlp:
            xTlt = xtlp.tile([P, DT, W], f32)
            nc.sync.dma_start(out=r32(xTlt[:, :, :]),
                              in_=r32(xTl.rearrange("(t p) s -> p t s", p=P)))

            with tc.tile_pool(name="xt", bufs=1) as xtp:
                xTt = xtp.tile([P, DT, S], f32)
                nc.sync.dma_start(out=r32(xTt[:, :, :]),
                                  in_=r32(xT.rearrange("(t p) s -> p t s", p=P)))
                qkv_phase(xTt, qkvwT1, kT1s, v1s, qkb1sb, vb1sb, xTlt)

            if fl.m1:
                with tc.tile_pool(name="m1p", bufs=1) as m1p:
                    m1sb = m1p.tile([P, NT, W], f32)
                    nc.sync.dma_start(out=m1sb[:, :, :],
                                      in_=m1.rearrange("n p w -> p n w"))
                    attn_phase(kT1s, v1s, m1sb, False)
            else:
                attn_phase(kT1s, v1s, None, False)

            proj_resid_ln(owT1, ob1sb, xTlt, lns["g1"], lns["b1"], x1T)

        # ---- x1 all-gather within batch group ----
        with tc.tile_pool(name="xf", bufs=1) as xfp:
            if cfg.use_collective:
                nc.sync.dma_start(
                    out=xb[:, :].rearrange("(t p) w -> p t w", p=P),
                    in_=x1T[:, :, :],
                )
                nc.gpsimd.collective_compute(
                    "AllGather",
                    bass.mybir.AluOpType.bypass,
                    replica_groups=[[0, 1, 2, 3], [4, 5, 6, 7]],
                    ins=[xb[:, :]],
                    outs=[agx[:, :]],
                )
                x1fT = xfp.tile([P, DT, S], f32)
                ag4 = agx[:, :].rearrange("(g t p) w -> g t p w", g=4, p=P)
                for dt in range(DT):
                    nc.sync.dma_start(
                        out=r32(x1fT[:, dt, :].rearrange("p (g w) -> p g w", g=4)),
                        in_=r32(ag4[:, dt, :, :].rearrange("g p w -> p g w")),
                    )
            else:
                raise NotImplementedError("non-collective fallback not built")

            qkv_phase(x1fT, qkvwT2, kT2s, v2s, qkb2sb, vb2sb, x1T)

        attn_phase(kT2s, v2s, None, fl.kb2)

        late = es.enter_context(tc.tile_pool(name="late", bufs=1))
        x2T = late.tile([P, DT, W], f32)
        proj_resid_ln(owT2, ob2sb, x1T, lns["g2"], lns["b2"], x2T)

        # ================= FFN =================
        with tc.tile_pool(name="ffh", bufs=1) as fhp, \
             tc.tile_pool(name="ffw", bufs=3) as wp, \
             tc.tile_pool(name="fft", bufs=2) as lp, \
             tc.tile_pool(name="ffpre", bufs=1) as prep, \
             tc.tile_pool(name="ffps", bufs=2, space="PSUM") as psp, \
             tc.tile_pool(name="fflnps", bufs=1, space="PSUM") as lpp:
            hT = fhp.tile([P, FT, W], f32)
            for f in range(FT):
                ps = psp.tile([P, W], f32, tag="f1ps")
                for dt in range(DT):
                    wsb = wp.tile([P, P], f32, tag="f1w")
                    nc.sync.dma_start(
                        out=r32(wsb[:, :]),
                        in_=r32(w1T[dt * P : (dt + 1) * P, f * P : (f + 1) * P]),
                    )
                    nc.tensor.matmul(
                        ps[:, :], lhsT=r32(wsb[:, :]), rhs=r32(x2T[:, dt, :]),
                        start=(dt == 0), stop=(dt == DT - 1),
                    )
                if fb1sb is not None:
                    nc.scalar.activation(out=r32(hT[:, f, :]), in_=ps[:, :],
                                         func=AF.Relu,
                                         bias=fb1sb[:, f : f + 1], scale=1.0)
                else:
                    nc.scalar.activation(out=r32(hT[:, f, :]), in_=ps[:, :],
                                         func=AF.Relu)
            pre = prep.tile([P, DT, W], f32, tag="ffpre")
            for d in range(DT):
                ps = psp.tile([P, W], f32, tag="f2ps")
                for ft in range(FT):
                    wsb = wp.tile([P, P], f32, tag="f2w")
                    nc.sync.dma_start(
                        out=r32(wsb[:, :]),
                        in_=r32(w2T[ft * P : (ft + 1) * P, d * P : (d + 1) * P]),
                    )
                    nc.tensor.matmul(
                        ps[:, :], lhsT=r32(wsb[:, :]), rhs=r32(hT[:, ft, :]),
                        start=(ft == 0), stop=(ft == FT - 1),
                    )
                if fb2sb is not None:
                    tmp = lp.tile([P, W], f32, tag="f2tmp")
                    nc.scalar.activation(out=tmp[:, :], in_=ps[:, :],
                                         func=AF.Identity,
                                         bias=fb2sb[:, d : d + 1], scale=1.0)
                    nc.vector.tensor_add(pre[:, d, :], tmp[:, :], x2T[:, d, :])
                else:
                    nc.vector.tensor_add(pre[:, d, :], ps[:, :], x2T[:, d, :])
            outT = fhp.tile([P, DT, W], f32, tag="outT")
            ln_t(pre, outT, lns["g3"], lns["b3"], lpp, lp, round_out=False)
            for d in range(DT):
                nc.sync.dma_start(out=out[d * P : (d + 1) * P, :],
                                  in_=outT[:, d, :])


def make_program(cfg, fl):
    from concourse import bacc
    import concourse.tile as tile

    nc = bacc.Bacc("TRN2", target_bir_lowering=False, debug=False,
                   num_devices=8)
    with tile.TileContext(nc) as tc:
        _build(nc, tc, cfg, fl)
    nc.compile()
    return nc


def prep_inputs(inputs, cfg):
    """Host-side data prep. Returns (in_maps, fl)."""
    B, S, D, H, DFF, W, NT = (cfg.B, cfg.S, cfg.D, cfg.H, cfg.DFF,
                              cfg.W, cfg.NT)
    f = np.float32
    x = np.asarray(inputs["x"], f)
    enc = np.asarray(inputs["enc_out"])
    trg = np.asarray(inputs["trg_mask"])
    fl = Flags()
    fl.qkb1 = bool(np.any(inputs["qkv_b1"]))
    fl.qkb2 = bool(np.any(inputs["qkv_b2"]))
    fl.vb1 = bool(np.any(np.asarray(inputs["qkv_b1"])[2 * D :]))
    fl.vb2 = bool(np.any(np.asarray(inputs["qkv_b2"])[2 * D :]))
    fl.ob1 = bool(np.any(inputs["out_b1"]))
    fl.ob2 = bool(np.any(inputs["out_b2"]))
    fl.fb1 = bool(np.any(inputs["ff_b1"]))
    fl.fb2 = bool(np.any(inputs["ff_b2"]))
    fl.g1 = not bool(np.all(np.asarray(inputs["ln1_g"]) == 1))
    fl.b1 = bool(np.any(inputs["ln1_b"]))
    fl.g2 = not bool(np.all(np.asarray(inputs["ln2_g"]) == 1))
    fl.b2 = bool(np.any(inputs["ln2_b"]))
    fl.g3 = not bool(np.all(np.asarray(inputs["ln3_g"]) == 1))
    fl.b3 = bool(np.any(inputs["ln3_b"]))
    fl.m1 = not bool(np.all(trg != 0))
    fl.kb2 = bool(np.any(enc == 0))

    shared = {
        "qkvwT1": np.ascontiguousarray(np.asarray(inputs["qkv_w1"], f).T),
        "qkvwT2": np.ascontiguousarray(np.asarray(inputs["qkv_w2"], f).T),
        "owT1": np.ascontiguousarray(np.asarray(inputs["out_w1"], f).T),
        "owT2": np.ascontiguousarray(np.asarray(inputs["out_w2"], f).T),
        "w1T": np.ascontiguousarray(np.asarray(inputs["ff_w1"], f).T),
        "w2T": np.ascontiguousarray(np.asarray(inputs["ff_w2"], f).T),
    }
    if fl.qkb1:
        shared["qkvb1"] = np.asarray(inputs["qkv_b1"], f)
    if fl.qkb2:
        shared["qkvb2"] = np.asarray(inputs["qkv_b2"], f)
    if fl.vb1:
        shared["vb1"] = np.broadcast_to(
            np.asarray(inputs["qkv_b1"], f)[2 * D :], (P, D)).copy()
    if fl.vb2:
        shared["vb2"] = np.broadcast_to(
            np.asarray(inputs["qkv_b2"], f)[2 * D :], (P, D)).copy()
    if fl.ob1:
        shared["ob1"] = np.asarray(inputs["out_b1"], f)
    if fl.ob2:
        shared["ob2"] = np.asarray(inputs["out_b2"], f)
    if fl.fb1:
        shared["fb1"] = np.asarray(inputs["ff_b1"], f)
    if fl.fb2:
        shared["fb2"] = np.asarray(inputs["ff_b2"], f)
    for nm, key, use in [("g1", "ln1_g", fl.g1), ("b1", "ln1_b", fl.b1),
                         ("g2", "ln2_g", fl.g2), ("b2", "ln2_b", fl.b2),
                         ("g3", "ln3_g", fl.g3), ("b3", "ln3_b", fl.b3)]:
        if use:
            shared[nm] = np.asarray(inputs[key], f)

    xTb = [np.ascontiguousarray(x[b].T) for b in range(B)]
    in_maps = []
    for c in range(8):
        b, r = c // 4, c % 4
        m = dict(shared)
        m["xT"] = xTb[b]
        m["xTl"] = np.ascontiguousarray(xTb[b][:, r * W : (r + 1) * W])
        if fl.m1:
            # m1[kt, i, j] = trg[0or b, 0, r*W + j, kt*P + i]  (0/1 float)
            tb = trg[b] if trg.shape[0] == B else trg[0]
            blk = tb[0, r * W : (r + 1) * W, :]  # [W, S] (q, k)
            m["m1"] = np.ascontiguousarray(
                (blk.T != 0).astype(f).reshape(NT, P, W))
        if fl.kb2:
            eb = enc[b, 0, 0, :]  # [S]
            m["kb2"] = np.where(eb != 0, f(0.0), f(-1e20)).astype(f).reshape(
                NT, P, 1)
        in_maps.append(m)
    return in_maps, fl


def kernel_with_results(**inputs):
    from concourse.bass_utils import run_bass_kernel_spmd

    cfg = Cfg()
    x = np.asarray(inputs["x"])
    assert x.shape == (cfg.B, cfg.S, cfg.D), x.shape
    in_maps, fl = prep_inputs(inputs, cfg)
    nc = make_program(cfg, fl)
    res = run_bass_kernel_spmd(nc, in_maps, list(range(8)))
    y = np.empty((cfg.B, cfg.S, cfg.D), np.float32)
    for c in range(8):
        b, r = c // 4, c % 4
        y[b, r * cfg.W : (r + 1) * cfg.W, :] = res.results[c]["out"].T
    return y, res


def kernel(**inputs):
    return kernel_with_results(**inputs)[0]
